# revision 8
# baseline (speedup 1.0000x reference)
"""Trainium2 Bass kernel for nn_AttnResLayer (sparse_attention).

Computes, for V [N=12, B=4, T=2048, D=1024] fp32:
  K = rmsnorm(V) * norm_weight
  logits[n,b,t] = dot(w_l, K[n,b,t,:])
  alpha = softmax(logits, axis=n)
  out[b,t,d] = sum_n alpha[n,b,t] * V[n,b,t,d]

Sharding: T split across 8 cores (256 tokens/core per b); w_l/norm_weight
replicated (folded into one weight vector host-side). No collectives.

Per-core kernel (per 128-token chunk, natural layout [128 tok, 1024 d]):
  - sum_d V^2 via ScalarE Square activation with fused accum_out
  - sum_d w*V via VectorE tensor_tensor_reduce (fused mult+reduce)
  - rms = exp(-0.5*ln(ss/D + eps)) on ScalarE (one table set with softmax exp)
  - softmax over n (free dim, 12) with fused exp+sum
  - out = sum_n diag(alpha_n) @ V_n on TensorE, accumulated in PSUM
"""

import numpy as np
from contextlib import ExitStack

import concourse.bass as bass
import concourse.bacc as bacc
import concourse.tile as tile
from concourse import mybir
from concourse.bass_utils import run_bass_kernel_spmd

N, B, T, D = 12, 4, 2048, 1024
NCORES = 8
TSH = T // NCORES  # tokens per core (per b)
P = 128            # tokens per chunk (partition dim)
NCHUNK = TSH // P
EPS = 1e-6
FP32 = mybir.dt.float32
AF = mybir.ActivationFunctionType
ALU = mybir.AluOpType
H = D // 512       # matmul moving-operand free-dim limit is 512 fp32


def _build_nc() -> bacc.Bacc:
    nc = bacc.Bacc("TRN2", target_bir_lowering=False, debug=False,
                   num_devices=NCORES)
    v_in = nc.dram_tensor("v", [N, B, TSH, D], FP32, kind="ExternalInput").ap()
    wb_in = nc.dram_tensor("wb", [P, D], FP32, kind="ExternalInput").ap()
    id_in = nc.dram_tensor("ident", [P, P], FP32, kind="ExternalInput").ap()
    out_d = nc.dram_tensor("out", [B, TSH, D], FP32, kind="ExternalOutput").ap()

    with tile.TileContext(nc) as tc, ExitStack() as ctx:
        const_pool = ctx.enter_context(tc.tile_pool(name="const", bufs=1))
        v_pool = ctx.enter_context(tc.tile_pool(name="vp", bufs=24))
        scr_pool = ctx.enter_context(tc.tile_pool(name="scr", bufs=1))
        small_pool = ctx.enter_context(tc.tile_pool(name="small", bufs=3))
        diag_pool = ctx.enter_context(tc.tile_pool(name="diag", bufs=24))
        psum_pool = ctx.enter_context(
            tc.tile_pool(name="accp", bufs=2, space="PSUM"))
        out_pool = ctx.enter_context(tc.tile_pool(name="outp", bufs=2))

        wb_t = const_pool.tile([P, D], FP32, name="wb_t")
        nc.sync.dma_start(wb_t[:], wb_in[:])
        id_t = const_pool.tile([P, P], FP32, name="id_t")
        nc.sync.dma_start(id_t[:], id_in[:])
        scr_act = scr_pool.tile([P, D], FP32, name="scr_act")
        scr_dve = scr_pool.tile([P, D], FP32, name="scr_dve")
        eps_t = const_pool.tile([P, 1], FP32, name="eps_t")
        nc.vector.memset(eps_t[:], EPS)

        for b in range(B):
            for c in range(NCHUNK):
                t0 = c * P
                vts = []
                for n in range(N):
                    vt = v_pool.tile([P, D], FP32, name="vt", tag="vt")
                    nc.sync.dma_start(vt[:], v_in[n, b, t0:t0 + P, :])
                    vts.append(vt)

                ss = small_pool.tile([P, N], FP32, name="ss", tag="ss")
                dot = small_pool.tile([P, N], FP32, name="dot", tag="dot")
                for n in range(N):
                    nc.scalar.activation(scr_act[:], vts[n][:], AF.Square,
                                         accum_out=ss[:, n:n + 1])
                    nc.vector.scalar_tensor_tensor(
                        out=scr_dve[:], in0=vts[n][:], scalar=0.0,
                        in1=wb_t[:], op0=ALU.bypass, op1=ALU.mult,
                        accum_out=dot[:, n:n + 1])

                # rms = (mean(V^2) + eps)^-0.5 = exp(-0.5*ln(ss/D + eps))
                u = small_pool.tile([P, N], FP32, name="u", tag="u")
                nc.scalar.activation(u[:], ss[:], AF.Ln, bias=eps_t[:, 0:1],
                                     scale=1.0 / D)
                rms = small_pool.tile([P, N], FP32, name="rms", tag="rms")
                nc.scalar.activation(rms[:], u[:], AF.Exp, scale=-0.5)
                logits = small_pool.tile([P, N], FP32, name="logits", tag="lg")
                nc.vector.tensor_mul(logits[:], dot[:], rms[:])

                # softmax over n (free dim): exp(x - max) fused with sum
                negmax = small_pool.tile([P, 1], FP32, name="negmax", tag="nm")
                nc.vector.tensor_reduce(negmax[:], logits[:],
                                        axis=mybir.AxisListType.X,
                                        op=ALU.max, negate=True)
                aexp = small_pool.tile([P, N], FP32, name="aexp", tag="ax")
                sumexp = small_pool.tile([P, 1], FP32, name="sumexp", tag="se")
                nc.scalar.activation(aexp[:], logits[:], AF.Exp,
                                     bias=negmax[:, 0:1], accum_out=sumexp[:])
                recip = small_pool.tile([P, 1], FP32, name="recip", tag="rc")
                nc.vector.reciprocal(recip[:], sumexp[:])

                # diag(alpha_n) tiles; normalization folded in
                dgs = []
                for n in range(N):
                    dg = diag_pool.tile([P, P], FP32, name="dg", tag="dg")
                    nc.vector.tensor_scalar(out=dg[:], in0=id_t[:],
                                            scalar1=aexp[:, n:n + 1],
                                            scalar2=recip[:, 0:1],
                                            op0=ALU.mult, op1=ALU.mult)
                    dgs.append(dg)

                # out[t, d] = sum_n alpha[n, t] * V_n[t, d] on TensorE
                acc = psum_pool.tile([P, D], FP32, name="acc", tag="acc")
                for h in range(H):
                    for n in range(N):
                        nc.tensor.matmul(acc[:, h * 512:(h + 1) * 512],
                                         dgs[n][:],
                                         vts[n][:, h * 512:(h + 1) * 512],
                                         start=(n == 0), stop=(n == N - 1))
                out_sb = out_pool.tile([P, D], FP32, name="out_sb", tag="ot")
                nc.scalar.copy(out_sb[:], acc[:])
                nc.sync.dma_start(out_d[b, t0:t0 + P, :], out_sb[:])
    nc.compile()
    return nc


_NC = None


def _get_nc() -> bacc.Bacc:
    global _NC
    if _NC is None:
        _NC = _build_nc()
    return _NC


def _make_in_maps(V, w_l, norm_weight):
    V = np.ascontiguousarray(np.asarray(V, dtype=np.float32))
    w = np.asarray(w_l, np.float32) * np.asarray(norm_weight, np.float32)
    wb = np.ascontiguousarray(np.broadcast_to(w, (P, D)))
    ident = np.eye(P, dtype=np.float32)
    in_maps = []
    for c in range(NCORES):
        vs = np.ascontiguousarray(V[:, :, c * TSH:(c + 1) * TSH, :])
        in_maps.append({"v": vs, "wb": wb, "ident": ident})
    return in_maps


def _run(in_maps, trace=False, **kwargs):
    return run_bass_kernel_spmd(_get_nc(), in_maps, list(range(NCORES)),
                                trace=trace, **kwargs)


def kernel(V, w_l, norm_weight):
    res = _run(_make_in_maps(V, w_l, norm_weight))
    outs = [res.results[i]["out"] for i in range(NCORES)]
    return np.concatenate(outs, axis=1).astype(np.float32)


# revision 18
# speedup vs baseline: 1.3855x; 1.3855x over previous
"""Trainium2 Bass kernel for nn_AttnResLayer (sparse_attention).

Computes, for V [N=12, B=4, T=2048, D=1024] fp32:
  K = rmsnorm(V) * norm_weight
  logits[n,b,t] = dot(w_l, K[n,b,t,:])
  alpha = softmax(logits, axis=n)
  out[b,t,d] = sum_n alpha[n,b,t] * V[n,b,t,d]

Sharding: T split across 8 cores (256 tokens/core per b); w_l/norm_weight
replicated (folded into one weight vector host-side). No collectives.

Per-core kernel (per 128-token chunk, natural layout [128 tok, 1024 d]):
  - sum_d V^2 via ScalarE Square activation with fused accum_out
  - sum_d w*V via VectorE tensor_tensor_reduce (fused mult+reduce)
  - rms = exp(-0.5*ln(ss/D + eps)) on ScalarE (one table set with softmax exp)
  - softmax over n (free dim, 12) with fused exp+sum
  - out = sum_n diag(alpha_n) @ V_n on TensorE, accumulated in PSUM
"""

import numpy as np
from contextlib import ExitStack

import concourse.bass as bass
import concourse.bacc as bacc
import concourse.tile as tile
from concourse import mybir
from concourse.bass_utils import run_bass_kernel_spmd

# Pin all activations to the one table set containing exp+ln+square so the
# compiler emits a single ACT_TABLE_LOAD instead of thrashing sets per chunk.
_orig_get_tables = bacc.get_activation_tables


def _pinned_tables(arch):
    tables = _orig_get_tables(arch)
    keep = "natural_log_exp_and_others"
    return {k: (v if k == keep else set()) for k, v in tables.items()}


bacc.get_activation_tables = _pinned_tables

N, B, T, D = 12, 4, 2048, 1024
NCORES = 8
TSH = T // NCORES  # tokens per core (per b)
P = 128            # tokens per chunk (partition dim)
NCHUNK = TSH // P
EPS = 1e-6
FP32 = mybir.dt.float32
FP32R = mybir.dt.float32r
AF = mybir.ActivationFunctionType
ALU = mybir.AluOpType
H = D // 512       # matmul moving-operand free-dim limit is 512 fp32


def _build_nc() -> bacc.Bacc:
    nc = bacc.Bacc("TRN2", target_bir_lowering=False, debug=False,
                   num_devices=NCORES)
    v_in = nc.dram_tensor("v", [N, B, TSH, D], FP32R, kind="ExternalInput").ap()
    wb_in = nc.dram_tensor("wb", [P, D], FP32, kind="ExternalInput").ap()
    id_in = nc.dram_tensor("ident", [P, P], FP32, kind="ExternalInput").ap()
    out_d = nc.dram_tensor("out", [B, TSH, D], FP32, kind="ExternalOutput").ap()

    with tile.TileContext(nc) as tc, ExitStack() as ctx:
        const_pool = ctx.enter_context(tc.tile_pool(name="const", bufs=1))
        v_pool = ctx.enter_context(tc.tile_pool(name="vp", bufs=3))
        scr_pool = ctx.enter_context(tc.tile_pool(name="scr", bufs=1))
        small_pool = ctx.enter_context(tc.tile_pool(name="small", bufs=3))
        diag_pool = ctx.enter_context(tc.tile_pool(name="diag", bufs=16))
        psum_pool = ctx.enter_context(
            tc.tile_pool(name="accp", bufs=4, space="PSUM"))
        out_pool = ctx.enter_context(tc.tile_pool(name="outp", bufs=3))

        wb_t = const_pool.tile([P, D], FP32, name="wb_t")
        nc.sync.dma_start(wb_t[:], wb_in[:])
        id_t = const_pool.tile([P, P], FP32, name="id_t")
        nc.sync.dma_start(id_t[:], id_in[:])
        scr_act = scr_pool.tile([P, D], FP32, name="scr_act")
        scr_dve = scr_pool.tile([P, D], FP32, name="scr_dve")
        scr_gp = scr_pool.tile([P, D], FP32, name="scr_gp")
        eps_t = const_pool.tile([P, 1], FP32, name="eps_t")
        nc.vector.memset(eps_t[:], EPS)

        for b in range(B):
            for c in range(NCHUNK):
                t0 = c * P
                vblk = v_pool.tile([P, N, D], FP32R, name="vblk", tag="vblk")
                nc.sync.dma_start(
                    vblk[:], v_in[:, b, t0:t0 + P, :].rearrange("n t d -> t n d"))
                vts = [vblk[:, n, :].bitcast(FP32) for n in range(N)]
                vts_r = [vblk[:, n, :] for n in range(N)]

                ss = small_pool.tile([P, N], FP32, name="ss", tag="ss")
                dot = small_pool.tile([P, N], FP32, name="dot", tag="dot")
                # gpsimd can't run TensorScalarPtr: squares on ACT, dots on DVE
                for n in range(N):
                    nc.scalar.activation(scr_act[:], vts[n], AF.Square,
                                         accum_out=ss[:, n:n + 1])
                for n in range(N):
                    nc.vector.scalar_tensor_tensor(
                        out=scr_dve[:], in0=vts[n], scalar=0.0,
                        in1=wb_t[:], op0=ALU.bypass, op1=ALU.mult,
                        accum_out=dot[:, n:n + 1])

                # rms = (mean(V^2) + eps)^-0.5 = exp(-0.5*ln(ss/D + eps))
                u = small_pool.tile([P, N], FP32, name="u", tag="u")
                nc.scalar.activation(u[:], ss[:], AF.Ln, bias=eps_t[:, 0:1],
                                     scale=1.0 / D)
                rms = small_pool.tile([P, N], FP32, name="rms", tag="rms")
                nc.scalar.activation(rms[:], u[:], AF.Exp, scale=-0.5)
                logits = small_pool.tile([P, N], FP32, name="logits", tag="lg")
                nc.vector.tensor_mul(logits[:], dot[:], rms[:])

                # softmax over n (free dim): exp(x - max) fused with sum
                negmax = small_pool.tile([P, 1], FP32, name="negmax", tag="nm")
                nc.vector.tensor_reduce(negmax[:], logits[:],
                                        axis=mybir.AxisListType.X,
                                        op=ALU.max, negate=True)
                aexp = small_pool.tile([P, N], FP32, name="aexp", tag="ax")
                sumexp = small_pool.tile([P, 1], FP32, name="sumexp", tag="se")
                nc.scalar.activation(aexp[:], logits[:], AF.Exp,
                                     bias=negmax[:, 0:1], accum_out=sumexp[:])
                recip = small_pool.tile([P, 1], FP32, name="recip", tag="rc")
                nc.vector.reciprocal(recip[:], sumexp[:])

                # diag(alpha_n) tiles; normalization folded in
                dgs = []
                for n in range(N):
                    dg = diag_pool.tile([P, P], FP32R, name="dg", tag="dg")
                    nc.vector.tensor_scalar(out=dg[:], in0=id_t[:],
                                            scalar1=aexp[:, n:n + 1],
                                            scalar2=recip[:, 0:1],
                                            op0=ALU.mult, op1=ALU.mult)
                    dgs.append(dg)

                # out[t, d] = sum_n alpha[n, t] * V_n[t, d] on TensorE
                acc = psum_pool.tile([P, D], FP32, name="acc", tag="acc")
                for h in range(H):
                    for n in range(N):
                        nc.tensor.matmul(acc[:, h * 512:(h + 1) * 512],
                                         dgs[n][:],
                                         vts_r[n][:, h * 512:(h + 1) * 512],
                                         start=(n == 0), stop=(n == N - 1))
                out_sb = out_pool.tile([P, D], FP32, name="out_sb", tag="ot")
                # Demote the PSUM drain + store so they don't block the next
                # chunk's work in the in-order ACT/SP queues.
                with tc.high_priority(offset=-270):
                    nc.scalar.copy(out_sb[:], acc[:])
                    nc.sync.dma_start(out_d[b, t0:t0 + P, :], out_sb[:])
    nc.compile()
    return nc


_NC = None


def _get_nc() -> bacc.Bacc:
    global _NC
    if _NC is None:
        _NC = _build_nc()
    return _NC


def _make_in_maps(V, w_l, norm_weight):
    V = np.ascontiguousarray(np.asarray(V, dtype=np.float32))
    w = np.asarray(w_l, np.float32) * np.asarray(norm_weight, np.float32)
    wb = np.ascontiguousarray(np.broadcast_to(w, (P, D)))
    ident = np.eye(P, dtype=np.float32)
    in_maps = []
    for c in range(NCORES):
        vs = np.ascontiguousarray(V[:, :, c * TSH:(c + 1) * TSH, :])
        in_maps.append({"v": vs, "wb": wb, "ident": ident})
    return in_maps


def _run(in_maps, trace=False, **kwargs):
    return run_bass_kernel_spmd(_get_nc(), in_maps, list(range(NCORES)),
                                trace=trace, **kwargs)


def kernel(V, w_l, norm_weight):
    res = _run(_make_in_maps(V, w_l, norm_weight))
    outs = [res.results[i]["out"] for i in range(NCORES)]
    return np.concatenate(outs, axis=1).astype(np.float32)
